# revision 1
# baseline (speedup 1.0000x reference)
"""Trainium2 Bass kernel for nn_ButterflyLayer2D (butterfly 2D CNN).

Strategy: pure data parallel over 8 NeuronCores (16 batch each), with the
per-core batch processed in 2 phases of 8 to fit SBUF.

All tensors are pre-arranged on the host (numpy) into DMA-friendly layouts:
  - activations live in SBUF as [128 = (w%2)*64 + c, (node, b, h, w//2)]
    so each 2x2-stride-2 per-node conv becomes 4 fp32r matmuls with K=128
    chunks: col-group q = output w-parity (tile_position (0, 64q)), x-chunks
    accumulate in PSUM. PSUM [128=(q,c_out), N] is evicted full-width by a
    single relu+bias op (alternating ScalarE/VectorE) directly into the next
    level's interleaved layout — zero data reshuffling anywhere on chip.
  - the input 4x4-patch conv uses the same trick with K=16 row-groups spread
    over 4 partition groups (one per b%4) for PE concurrency.
  - the final per-node dense is a [64,128] x [64,b] matmul; outputs are
    written as [128=(r,ou,ov), (ph,node,b)] and decoded on the host.
Weights are streamed from HBM in 8-node chunks through a recycled tile tag.
"""

import numpy as np
from contextlib import ExitStack

import concourse.bass as bass
import concourse.tile as tile
from concourse import bacc, mybir
from concourse.bass_utils import run_bass_kernel_spmd

F32 = mybir.dt.float32
F32R = mybir.dt.float32r
BF16 = mybir.dt.bfloat16
AF = mybir.ActivationFunctionType
ALU = mybir.AluOpType

B, IN, NLVL, KLVL, C = 128, 256, 6, 3, 64
TCOL = 1024               # psum tile columns
PBUFS = 4                 # psum tile bufs
NK, OU, OV = 8, 8, 8
NCORES = 8
BC = B // NCORES          # 16 per-core batch
PH = 1                    # phases per core
BG = BC // PH             # batch per phase
HALF = BG // 4            # input-conv b-subgroups per partition group
LVL_NODES = [4, 16, 64, 64, 64, 64]          # nodes per level
LVL_HIN = [64, 32, 16, 8, 4, 2]              # spatial H into each level
WGRP = 8                  # weight streaming chunk (nodes)


# ----------------------------------------------------------------------------
# host-side pre-arrangement
# ----------------------------------------------------------------------------

def _prep_weights(inputs):
    """Weights/biases blobs shared by all cores."""
    out = {}
    # input filter: lhsT [16=(p,q), 64], replicated at partition bases 0/32/64/96
    import ml_dtypes
    fin = inputs["in_filter"][:, :, 0, :].reshape(16, C).astype(np.float32)
    finr = np.zeros((128, C), np.float32)
    for g in range(4):
        finr[g * 32 : g * 32 + 16] = fin
    out["fin"] = finr.astype(ml_dtypes.bfloat16)
    out["bin"] = np.concatenate([inputs["in_bias"], inputs["in_bias"]]).reshape(
        128, 1
    ).astype(np.float32)

    for lvl in range(1, NLVL + 1):
        f = inputs[f"f{lvl}"].astype(np.float32)  # [n,n,2,2,C,C] (x,y,ci,co)
        n = f.shape[0]
        assert n == 2 ** min(lvl, KLVL)
        # lhsT per node: [(y*64+ci), (x*64+co)]
        w = f.transpose(0, 1, 3, 4, 2, 5).reshape(n * n, 2 * C, 2 * C)
        if lvl <= KLVL:
            # sibling-pair blob: per pair (u,2t)+(u,2t+1):
            # [(y,ci)=128, (x, coA|coB)=256] -> [128, pairs*256]
            wp = w.reshape(n * n // 2, 2, 2 * C, 2, C)  # [pair, s, (y,ci), x, co]
            wp = wp.transpose(2, 0, 3, 1, 4)            # [(y,ci), pair, x, s, co]
            out[f"w{lvl}"] = np.ascontiguousarray(wp).reshape(
                128, n * n * 128
            ).astype(ml_dtypes.bfloat16)
        else:
            # blob [128, nodes*128], free = (node, x*64+co)
            out[f"w{lvl}"] = np.ascontiguousarray(w.transpose(1, 0, 2)).reshape(
                128, n * n * 128
            ).astype(ml_dtypes.bfloat16)
        b = inputs[f"b{lvl}"].astype(np.float32).reshape(n * n, C)
        if lvl < NLVL:
            # [128, nodes]: rows (q,c) with bias duplicated across q
            bb = np.concatenate([b, b], axis=1)  # [nodes, 128]
            out[f"b{lvl}"] = np.ascontiguousarray(bb.T)
        else:
            # lvl6 node-pair scheme: psum rows = (cA, cB) for pair (2k, 2k+1)
            bb = b.reshape(n * n // 2, 2 * C)  # [pairs, (cA,cB)]
            out[f"b{lvl}"] = np.ascontiguousarray(bb.T)  # [128, 32]
    # dense: lhsT per node [64=c, 128=(r, ou*8+ov)]
    wd = inputs["Wd"].astype(np.float32).reshape(NK * NK, 2, C, OU * OV)
    wd = wd.transpose(2, 0, 1, 3).reshape(C, NK * NK * 2 * OU * OV)
    out["wd"] = np.ascontiguousarray(wd).astype(ml_dtypes.bfloat16)
    return out


def _prep_input(in_data_core):
    """Per-core input blob: [64 = (b%4)*16 + (i%4)*4 + (j%4),
    (ph, b//4%2, x=i//4, y4=j//4)] packed (no zero rows)."""
    ind = in_data_core[:, :, :, 0]  # [16, 256, 256]
    a = ind.reshape(PH, HALF, 4, 64, 4, 64, 4)  # [ph, half, g, x, p, y4, q]
    a = a.transpose(2, 4, 6, 0, 1, 3, 5)        # [g, p, q, ph, half, x, y4]
    import ml_dtypes
    return np.ascontiguousarray(a).reshape(64, PH * HALF * 64 * 64).astype(ml_dtypes.bfloat16)


def _decode_output(t2_core):
    """t2 [128=(r,ou,ov), (ph, node, bl)] -> [16, 64, 64, 2]."""
    t = t2_core.reshape(2, OU, OV, PH, NK, NK, BG)  # r,ou,ov,ph,u,v,bl
    t = t.transpose(3, 6, 4, 1, 5, 2, 0)            # ph,bl,u,ou,v,ov,r
    return np.ascontiguousarray(t).reshape(BC, NK * OU, NK * OV, 2)


# ----------------------------------------------------------------------------
# device kernel
# ----------------------------------------------------------------------------

def _build_kernel(reps=1, xouter=True):
    nc = bacc.Bacc(None, target_bir_lowering=False)
    p = {}
    p["a0"] = nc.declare_dram_parameter("a0", [64, PH * HALF * 64 * 64], BF16, isOutput=False)
    p["fin"] = nc.declare_dram_parameter("fin", [128, C], BF16, isOutput=False)
    p["bin"] = nc.declare_dram_parameter("bin", [128, 1], F32, isOutput=False)
    for lvl in range(1, NLVL + 1):
        n2 = LVL_NODES[lvl - 1]
        p[f"w{lvl}"] = nc.declare_dram_parameter(f"w{lvl}", [128, n2 * 128], BF16, isOutput=False)
        bcols = n2 if lvl < NLVL else n2 // 2
        p[f"b{lvl}"] = nc.declare_dram_parameter(f"b{lvl}", [128, bcols], F32, isOutput=False)
    p["wd"] = nc.declare_dram_parameter("wd", [64, NK * NK * 128], BF16, isOutput=False)
    t2 = nc.declare_dram_parameter("t2", [128, PH * NK * NK * BG], F32, isOutput=True)

    evict_ctr = [0]

    def evict(out_ap, psum_ap, bias_ap):
        """relu(psum + bias) -> sbuf, alternating engines to split the load."""
        evict_ctr[0] += 1
        if evict_ctr[0] % 2 == 0:
            nc.scalar.activation(out_ap, psum_ap, AF.Relu, bias=bias_ap)
        else:
            nc.vector.tensor_scalar(out_ap, psum_ap, bias_ap, 0.0,
                                    op0=ALU.add, op1=ALU.max)

    with tile.TileContext(nc) as tc, ExitStack() as ctx:
        const = ctx.enter_context(tc.tile_pool(name="const", bufs=1))
        wpool = ctx.enter_context(tc.tile_pool(name="wts", bufs=4))
        apool = ctx.enter_context(tc.tile_pool(name="acts", bufs=1))
        inpool = ctx.enter_context(tc.tile_pool(name="inp", bufs=1))
        fpool = ctx.enter_context(tc.tile_pool(name="feat", bufs=2))
        ppool = ctx.enter_context(tc.tile_pool(name="ps", bufs=PBUFS, space="PSUM"))
        spool = ppool

        # constants: input filter, biases (all small, loaded once)
        fin_t = const.tile([128, C], BF16)
        nc.sync.dma_start(fin_t[:], p["fin"][:])
        bin_t = const.tile([128, 1], F32)
        nc.sync.dma_start(bin_t[:], p["bin"][:])
        bias_t = {}
        for lvl in range(1, NLVL + 1):
            bcols = LVL_NODES[lvl - 1] if lvl < NLVL else LVL_NODES[lvl - 1] // 2
            bias_t[lvl] = const.tile([128, bcols], F32, tag=f"bias{lvl}", name=f"bias{lvl}")
            nc.sync.dma_start(bias_t[lvl][:], p[f"b{lvl}"][:])

        for phx in range(reps * PH):
            ph = phx % PH
            # ---------------- input staging ----------------
            a0s = inpool.tile([128, HALF * 64 * 64], BF16, tag="a0s", name=f"a0s{phx}")
            for g in range(4):
                nc.sync.dma_start(
                    a0s[g * 32 : g * 32 + 16, :],
                    p["a0"][g * 16 : (g + 1) * 16,
                            ph * HALF * 64 * 64 : (ph + 1) * HALF * 64 * 64],
                )
            a0v = a0s[:].rearrange("p (h x y) -> p h x y", h=HALF, x=64)

            # ---------------- input conv ----------------
            # X slab: [128=(y%2,c), (bl, x, y2)]  (bl=8, x=64, y2=32)
            X = apool.tile([128, BG * 64 * 32], BF16, tag="s0", name=f"x{phx}")
            Xv = X[:].rearrange("p (b h w) -> p b h w", b=BG, h=64)
            for bl in range(BG):
                g, half = bl % 4, bl // 4
                for xh in range(2048 // TCOL):
                    pt = ppool.tile([128, TCOL], F32, tag="ps",
                                    padded_shape=[128, TCOL],
                                    name=f"pin{phx}_{bl}_{xh}")
                    for sub in range(TCOL // 512):
                        xq = xh * (TCOL // 512) + sub
                        for q in (0, 1):
                            rhs = a0v[g * 32 : g * 32 + 16, half,
                                      xq * 16 : (xq + 1) * 16, q::2]
                            nc.tensor.matmul(
                                pt[q * 64 : (q + 1) * 64,
                                   sub * 512 : (sub + 1) * 512],
                                fin_t[g * 32 : g * 32 + 16, :],
                                rhs,
                                start=True, stop=True,
                                tile_position=(g * 32, q * 64),
                            )
                    evict(Xv[:, bl, xh * (TCOL // 32) : (xh + 1) * (TCOL // 32), :], pt[:], bin_t[:, 0:1])

            # ---------------- levels 1..5 (q-scheme) ----------------
            cur = X          # slab with free = (node, bl, h, w2)
            cur_nodes = 1
            tags = ["s1", "s0", "s1", "s0", "s1"]
            for lvl in range(1, 6):
                n2 = LVL_NODES[lvl - 1]
                grid = int(np.sqrt(n2))
                Hin = LVL_HIN[lvl - 1]
                W2in = Hin // 2
                Ho, W2o = Hin // 2, W2in // 2
                ncols_out = BG * Ho * W2o
                nxt = apool.tile([128, n2 * ncols_out], BF16,
                                 tag=tags[lvl - 1], name=f"a{lvl}_{phx}")
                curv = cur[:].rearrange("p (n b h w) -> p n b h w",
                                        n=cur_nodes, b=BG, h=Hin)
                nxtv = nxt[:].rearrange("p (n b h w) -> p n b h w",
                                        n=n2, b=BG, h=Ho)
                pgrid = int(np.sqrt(cur_nodes))
                if lvl <= KLVL:
                    # sibling-pair scheme: M=128=(coA,coB), shared parent rhs
                    Wo = W2in          # output width = rhs w-count
                    npairs = n2 // 2
                    # block = (bper b, hper h, all Wo) == 1024 cols (2 banks)
                    hper = min(Ho, TCOL // Wo)
                    bper = min(BG, max(1, TCOL // (Wo * hper)))
                    ncol = bper * hper * Wo
                    PGRP = 4           # pairs per weight DMA chunk
                    for g0 in range(0, npairs, PGRP):
                        gn = min(PGRP, npairs - g0)
                        wlt = wpool.tile([128, PGRP * 256], BF16, tag="wch",
                                         name=f"w{lvl}_{phx}_{g0}")
                        nc.sync.dma_start(
                            wlt[:, : gn * 256],
                            p[f"w{lvl}"][:, g0 * 256 : (g0 + gn) * 256],
                        )
                        for pr in range(g0, g0 + gn):
                            u, t = pr // (grid // 2), pr % (grid // 2)
                            nA = u * grid + 2 * t
                            nB = nA + 1
                            lp_ = pr - g0
                            pnode = (u // 2) * pgrid + t
                            # sub-splitting along b (or h) into 512-col chunks
                            nsub = ncol // 512
                            if bper >= nsub:
                                sb, sh = bper // nsub, hper
                            else:
                                sb, sh = 1, hper // (nsub // max(1, bper))
                            hsubs = hper // sh
                            for bs in range(0, BG, bper):
                                for h0 in range(0, Ho, hper):
                                    pt = ppool.tile(
                                        [128, ncol], F32, tag="ps",
                                        padded_shape=[128, TCOL],
                                        name=f"p{lvl}_{phx}_{pr}_{bs}_{h0}")
                                    for sub in range(nsub):
                                        b1 = bs + (sub // hsubs) * sb
                                        h1 = h0 + (sub % hsubs) * sh
                                        for x in (0, 1):
                                            rhs = curv[:, pnode, b1 : b1 + sb,
                                                       2 * h1 + x : 2 * (h1 + sh) : 2,
                                                       :]
                                            nc.tensor.matmul(
                                                pt[:, sub * 512 : (sub + 1) * 512],
                                                wlt[:, lp_ * 256 + x * 128 :
                                                    lp_ * 256 + (x + 1) * 128],
                                                rhs,
                                                start=(x == 0), stop=(x == 1),
                                            )
                                    for shalf, node in ((0, nA), (1, nB)):
                                        ptv = pt[shalf * 64 : (shalf + 1) * 64, :] \
                                            .rearrange("c (b h w) -> c b h w",
                                                       b=bper, h=hper)
                                        bias_ap = bias_t[lvl][
                                            shalf * 64 : (shalf + 1) * 64,
                                            node : node + 1]
                                        for par in (0, 1):
                                            evict(
                                                nxtv[par * 64 : (par + 1) * 64,
                                                     node, bs : bs + bper,
                                                     h0 : h0 + hper, :],
                                                ptv[:, :, :, par::2],
                                                bias_ap,
                                            )
                else:
                    # q-scheme (deep levels)
                    nblk = max(1, ncols_out // 512)
                    bper = BG // nblk
                    ncol = bper * Ho * W2o
                    for g0 in range(0, n2, WGRP):
                        gn = min(WGRP, n2 - g0)
                        wlt = wpool.tile([128, WGRP * 128], BF16, tag="wch",
                                         name=f"w{lvl}_{phx}_{g0}")
                        nc.sync.dma_start(
                            wlt[:, : gn * 128],
                            p[f"w{lvl}"][:, g0 * 128 : (g0 + gn) * 128],
                        )
                        for node in range(g0, g0 + gn):
                            ln = node - g0
                            pnode = node
                            for blk in range(nblk):
                                bs = blk * bper
                                pt = ppool.tile([128, ncol], F32, tag="ps",
                                                padded_shape=[128, TCOL],
                                                name=f"p{lvl}_{phx}_{node}_{blk}")
                                qx = [(x, q) for x in (0, 1) for q in (0, 1)] \
                                    if xouter else \
                                    [(x, q) for q in (0, 1) for x in (0, 1)]
                                for x, q in qx:
                                    rhs = curv[:, pnode, bs : bs + bper, x::2, q::2]
                                    nc.tensor.matmul(
                                        pt[q * 64 : (q + 1) * 64, :],
                                        wlt[:, ln * 128 + x * 64 :
                                            ln * 128 + (x + 1) * 64],
                                        rhs,
                                        start=(x == 0), stop=(x == 1),
                                        skip_group_check=xouter,
                                        tile_position=(0, q * 64),
                                    )
                                evict(
                                    nxtv[:, node, bs : bs + bper, :, :],
                                    pt[:],
                                    bias_t[lvl][:, node : node + 1],
                                )
                cur = nxt
                cur_nodes = n2

            # ---------------- level 6 (node pairs, M=64) ----------------
            # cur: [128, (n=64, bl, h=2, w2=1)] ; feats F [64=c, (node, bl)]
            F = fpool.tile([64, NK * NK * BG], BF16, tag="feats", name=f"f{phx}")
            Fv = F[:].rearrange("c (n b) -> c n b", n=NK * NK)
            curv = cur[:].rearrange("p (n b h w) -> p n b h w", n=64, b=BG, h=2)
            for g0 in range(0, 64, WGRP):
                w6t = wpool.tile([128, WGRP * 128], BF16, tag="wch",
                                 name=f"w6_{phx}_{g0}")
                nc.sync.dma_start(
                    w6t[:], p["w6"][:, g0 * 128 : (g0 + WGRP) * 128]
                )
                for pr in range(g0 // 2, (g0 + WGRP) // 2):
                    nA, nB = 2 * pr, 2 * pr + 1
                    pt = spool.tile([128, BG], F32, tag="ps", padded_shape=[128, TCOL],
                                    name=f"p6_{phx}_{pr}")
                    hx = [(h_, x_) for x_ in (0, 1) for h_ in (0, 1)] \
                        if xouter else \
                        [(h_, x_) for h_ in (0, 1) for x_ in (0, 1)]
                    for half, x in hx:
                        node = nA if half == 0 else nB
                        ln = node - g0
                        rhs = curv[:, node, :, x, 0]
                        nc.tensor.matmul(
                            pt[half * 64 : (half + 1) * 64, :],
                            w6t[:, ln * 128 + x * 64 :
                                ln * 128 + (x + 1) * 64],
                            rhs,
                            start=(x == 0), stop=(x == 1),
                            skip_group_check=xouter,
                            tile_position=(0, half * 64),
                        )
                    bias_ap = bias_t[6][:, pr : pr + 1]
                    evict_ctr[0] += 1
                    if evict_ctr[0] % 2 == 0:
                        nc.scalar.activation(Fv[0:64, nA, :], pt[0:64, :], AF.Relu,
                                             bias=bias_ap[0:64, :])
                        nc.scalar.activation(Fv[0:64, nB, :], pt[64:128, :], AF.Relu,
                                             bias=bias_ap[64:128, :])
                    else:
                        nc.vector.tensor_scalar(Fv[0:64, nA, :], pt[0:64, :],
                                                bias_ap[0:64, :], 0.0,
                                                op0=ALU.add, op1=ALU.max)
                        nc.vector.tensor_scalar(Fv[0:64, nB, :], pt[64:128, :],
                                                bias_ap[64:128, :], 0.0,
                                                op0=ALU.add, op1=ALU.max)

            # ---------------- dense ----------------
            t2s = fpool.tile([128, NK * NK * BG], F32, tag="t2s", name=f"t2s{phx}")
            t2sv = t2s[:].rearrange("m (n b) -> m n b", n=NK * NK)
            for g0 in range(0, 64, WGRP):
                wdt = wpool.tile([64, WGRP * 128], BF16, tag="wdch",
                                 name=f"wd_{phx}_{g0}")
                nc.sync.dma_start(
                    wdt[:], p["wd"][:, g0 * 128 : (g0 + WGRP) * 128]
                )
                for node in range(g0, g0 + WGRP):
                    ln = node - g0
                    pt = spool.tile([128, BG], F32, tag="ps", padded_shape=[128, TCOL],
                                    name=f"pd_{phx}_{node}")
                    nc.tensor.matmul(
                        pt[:],
                        wdt[:, ln * 128 : (ln + 1) * 128],
                        Fv[:, node, :],
                        start=True, stop=True,
                    )
                    evict_ctr[0] += 1
                    if evict_ctr[0] % 2 == 0:
                        nc.scalar.copy(t2sv[:, node, :], pt[:])
                    else:
                        nc.vector.tensor_copy(t2sv[:, node, :], pt[:])
            nc.sync.dma_start(
                t2[:, ph * NK * NK * BG : (ph + 1) * NK * NK * BG], t2s[:]
            )
    nc.compile()
    return nc


# ----------------------------------------------------------------------------
# entry point
# ----------------------------------------------------------------------------

def kernel(**inputs):
    inputs = {k: np.asarray(v) for k, v in inputs.items()}
    wblobs = _prep_weights(inputs)
    nc = _build_kernel()
    in_maps = []
    for c in range(NCORES):
        m = dict(wblobs)
        m["a0"] = _prep_input(inputs["in_data"][c * BC : (c + 1) * BC])
        in_maps.append(m)
    res = run_bass_kernel_spmd(nc, in_maps, list(range(NCORES)))
    outs = [_decode_output(res.results[c]["t2"]) for c in range(NCORES)]
    return np.concatenate(outs, axis=0).astype(np.float32)


if __name__ == "__main__":
    import reference as ref

    inputs = {k: np.asarray(v) for k, v in ref.setup_inputs().items()}
    expected = np.asarray(ref.reference(**inputs))
    actual = kernel(**inputs)
    err = np.abs(actual - expected).max()
    rel = err / np.abs(expected).max()
    print("absmax:", err, "rel:", rel)



# revision 11
# speedup vs baseline: 1.3765x; 1.3765x over previous
"""Trainium2 Bass kernel for nn_ButterflyLayer2D (butterfly 2D CNN).

Strategy: pure data parallel over 8 NeuronCores (16 batch each).

Layouts (per core):
  - activations in SBUF as [128 = (w%2)*64 + c, (node, b, h, w//2)]; every
    2x2-stride-2 per-node conv is 4 bf16 matmuls with K=128=(y,ci):
    q = output w-parity goes to PE column-tile (0, q*64) so the two q
    streams run CONCURRENTLY on the two column halves of the PE array,
    x = input h-parity accumulates in PSUM.  PSUM rows are (q, c_out) ==
    exactly the next level's partition layout, so each eviction is ONE
    full-width relu+bias op with contiguous psum read and contiguous
    sbuf write (alternating ScalarE/VectorE).
  - the input 4x4-patch conv packs TWO horizontally adjacent patches per
    rhs column (K=32, block-diagonal lhsT, M=128=(w%2, c)) and row-tiles
    4 ways over (b%4) so four streams run concurrently and the staged
    input uses all 128 SBUF partitions (full DMA port bandwidth).
  - level 6 (spatial 1x1) computes node pairs with M=(coA,coB) into a
    single psum tile; features land as [128=(sib,c), (pair, b)].
  - the final dense is row-tiled 2x: even nodes on PE rows 0-63, odd on
    64-127, streaming concurrently.
Weights are prefetched whole-level (w1,w2,w4,w5,w6,wd) across both HWDGE
rings (sync + scalar); w3 streams just-in-time in 8-node chunks.
"""

import numpy as np
from contextlib import ExitStack

import concourse.bass as bass
import concourse.tile as tile
from concourse import bacc, mybir
from concourse.bass_utils import run_bass_kernel_spmd

F32 = mybir.dt.float32
BF16 = mybir.dt.bfloat16
AF = mybir.ActivationFunctionType
ALU = mybir.AluOpType

B, IN, NLVL, KLVL, C = 128, 256, 6, 3, 64
NK, OU, OV = 8, 8, 8
NCORES = 8
BC = B // NCORES          # 16 per-core batch
PH = 1
BG = BC
TCOL = 1024               # psum tile columns (2 banks)
PBUFS = 4
LVL_NODES = [4, 16, 64, 64, 64, 64]
LVL_HIN = [64, 32, 16, 8, 4, 2]


# ----------------------------------------------------------------------------
# host-side pre-arrangement
# ----------------------------------------------------------------------------

def _prep_weights(inputs):
    """Weights/biases blobs shared by all cores."""
    import ml_dtypes
    out = {}
    # input filter: block-diag lhsT [32=(yp,pix), 128=(yp_out, c)], repl x4
    fin = inputs["in_filter"][:, :, 0, :].reshape(16, C).astype(np.float32)
    blk = np.zeros((32, 128), np.float32)
    blk[0:16, 0:64] = fin
    blk[16:32, 64:128] = fin
    finr = np.zeros((128, 128), np.float32)
    for g in range(4):
        finr[g * 32 : (g + 1) * 32] = blk
    out["fin"] = finr.astype(ml_dtypes.bfloat16)
    out["bin"] = np.concatenate([inputs["in_bias"], inputs["in_bias"]]).reshape(
        128, 1
    ).astype(np.float32)

    for lvl in range(1, NLVL + 1):
        f = inputs[f"f{lvl}"].astype(np.float32)  # [n,n,2,2,C,C] (x,y,ci,co)
        n = f.shape[0]
        # per node lhsT [(y,ci)=128, (x,co)=128]
        w = f.transpose(0, 1, 3, 4, 2, 5).reshape(n * n, 2 * C, 2 * C)
        out[f"w{lvl}"] = np.ascontiguousarray(w.transpose(1, 0, 2)).reshape(
            128, n * n * 128
        ).astype(ml_dtypes.bfloat16)
        b = inputs[f"b{lvl}"].astype(np.float32).reshape(n * n, C)
        if lvl < NLVL:
            bb = np.concatenate([b, b], axis=1)  # [nodes, (q,c)=128]
            out[f"b{lvl}"] = np.ascontiguousarray(bb.T)
        else:
            bb = b.reshape(n * n // 2, 2 * C)    # [pairs, (cA,cB)]
            out[f"b{lvl}"] = np.ascontiguousarray(bb.T)  # [128, 32]
    # dense: [128, pairs*128]: rows 0-63 even-node [c,(r,ouov)], 64-127 odd
    wd = inputs["Wd"].astype(np.float32).reshape(NK * NK, 2, C, OU * OV)
    wd = wd.transpose(2, 0, 1, 3).reshape(C, NK * NK, 2 * OU * OV)
    wd2 = np.zeros((128, NK * NK // 2 * 128), np.float32)
    for k in range(NK * NK // 2):
        wd2[0:64, k * 128 : (k + 1) * 128] = wd[:, 2 * k]
        wd2[64:128, k * 128 : (k + 1) * 128] = wd[:, 2 * k + 1]
    out["wd"] = wd2.astype(ml_dtypes.bfloat16)
    return out


def _prep_input(in_data_core):
    """Per-core input blob [128 = (b%4)*32 + (j%8//4)*16 + (i%4)*4 + (j%4),
    (b//4, x=i//4, y2=j//8)]."""
    import ml_dtypes
    ind = in_data_core[:, :, :, 0]  # [16, 256, 256]
    a = ind.reshape(4, 4, 64, 4, 32, 2, 4)  # [half, g, x, p, y2, yp, q]
    a = a.transpose(1, 5, 3, 6, 0, 2, 4)    # [g, yp, p, q, half, x, y2]
    return np.ascontiguousarray(a).reshape(128, 4 * 64 * 32).astype(
        ml_dtypes.bfloat16
    )


def _decode_output(t2_core):
    """t2 [128=(r,ou,ov), (node, b)] -> [16, 64, 64, 2]."""
    t = t2_core.reshape(2, OU, OV, PH, NK, NK, BG)
    t = t.transpose(3, 6, 4, 1, 5, 2, 0)            # ph,b,u,ou,v,ov,r
    return np.ascontiguousarray(t).reshape(BC, NK * OU, NK * OV, 2)


# ----------------------------------------------------------------------------
# device kernel
# ----------------------------------------------------------------------------

def _build_kernel(reps=1, xouter=True):
    nc = bacc.Bacc(None, target_bir_lowering=False)
    p = {}
    p["a0"] = nc.declare_dram_parameter("a0", [128, 4 * 64 * 32], BF16, isOutput=False)
    p["fin"] = nc.declare_dram_parameter("fin", [128, 128], BF16, isOutput=False)
    p["bin"] = nc.declare_dram_parameter("bin", [128, 1], F32, isOutput=False)
    for lvl in range(1, NLVL + 1):
        n2 = LVL_NODES[lvl - 1]
        p[f"w{lvl}"] = nc.declare_dram_parameter(f"w{lvl}", [128, n2 * 128], BF16, isOutput=False)
        bcols = n2 if lvl < NLVL else n2 // 2
        p[f"b{lvl}"] = nc.declare_dram_parameter(f"b{lvl}", [128, bcols], F32, isOutput=False)
    p["wd"] = nc.declare_dram_parameter("wd", [128, NK * NK // 2 * 128], BF16, isOutput=False)
    t2 = nc.declare_dram_parameter("t2", [128, NK * NK * BG], F32, isOutput=True)

    evict_ctr = [0]

    def evict(out_ap, psum_ap, bias_ap):
        """relu(psum + bias) -> sbuf, alternating engines."""
        evict_ctr[0] += 1
        if evict_ctr[0] % 2 == 0:
            nc.scalar.activation(out_ap, psum_ap, AF.Relu, bias=bias_ap)
        else:
            nc.vector.tensor_scalar(out_ap, psum_ap, bias_ap, 0.0,
                                    op0=ALU.add, op1=ALU.max)

    with tile.TileContext(nc) as tc, ExitStack() as ctx:
        const = ctx.enter_context(tc.tile_pool(name="const", bufs=1))
        wbig = ctx.enter_context(tc.tile_pool(name="wbig", bufs=1))
        wpool = ctx.enter_context(tc.tile_pool(name="wts", bufs=2))
        apool = ctx.enter_context(tc.tile_pool(name="acts", bufs=1))
        inpool = ctx.enter_context(tc.tile_pool(name="inp", bufs=1))
        fpool = ctx.enter_context(tc.tile_pool(name="feat", bufs=1))
        ppool = ctx.enter_context(tc.tile_pool(name="ps", bufs=PBUFS, space="PSUM"))

        # ---------------- DMA prologue ----------------
        # sync ring: a0c0, consts, w1, w2, a0c1, w4
        # scalar ring: a0c2, a0c3, w5, w6, wd
        # a0 staging ping-pong: tile holds 2 of the 4 batch-halves at a time
        a0s = inpool.tile([128, 2 * 64 * 32], BF16, tag="a0s", name="a0s")
        nc.sync.dma_start(a0s[:, 0:2048], p["a0"][:, 0:2048])
        fin_t = const.tile([128, 128], BF16)
        nc.sync.dma_start(fin_t[:], p["fin"][:])
        bin_t = const.tile([128, 1], F32)
        nc.sync.dma_start(bin_t[:], p["bin"][:])
        bias_t = {}
        for lvl in range(1, NLVL + 1):
            bcols = LVL_NODES[lvl - 1] if lvl < NLVL else LVL_NODES[lvl - 1] // 2
            bias_t[lvl] = const.tile([128, bcols], F32, tag=f"bias{lvl}", name=f"bias{lvl}")
            nc.sync.dma_start(bias_t[lvl][:], p[f"b{lvl}"][:])
        w_t = {}
        for lvl, ring in ((1, nc.sync), (2, nc.sync)):
            n2 = LVL_NODES[lvl - 1]
            w_t[lvl] = wbig.tile([128, n2 * 128], BF16, tag=f"w{lvl}", name=f"w{lvl}")
            ring.dma_start(w_t[lvl][:], p[f"w{lvl}"][:])
        w_t[4] = wbig.tile([128, 64 * 128], BF16, tag="w4", name="w4")
        nc.sync.dma_start(w_t[4][:], p["w4"][:])
        nc.scalar.dma_start(a0s[:, 2048:4096], p["a0"][:, 2048:4096])
        for lvl in (5, 6):
            w_t[lvl] = wbig.tile([128, 64 * 128], BF16, tag=f"w{lvl}", name=f"w{lvl}")
            nc.scalar.dma_start(w_t[lvl][:], p[f"w{lvl}"][:])
        wd_t = wbig.tile([128, NK * NK // 2 * 128], BF16, tag="wd", name="wd")
        nc.scalar.dma_start(wd_t[:], p["wd"][:])

        a0v = a0s[:].rearrange("p (h x y) -> p h x y", h=2, x=64)

        # ---------------- input conv ----------------
        # X: [128=(w%2,c), (b, h=64, w2=32)]
        X = apool.tile([128, BG * 64 * 32], BF16, tag="s0", name="x0")
        X2d = X[:]
        for bl in range(BG):
            g, half = bl % 4, bl // 4
            if bl == 4:
                # stage half 2 over the slot bl 0-3 just finished reading
                nc.sync.dma_start(a0s[:, 0:2048], p["a0"][:, 4096:6144])
            elif bl == 8:
                nc.scalar.dma_start(a0s[:, 2048:4096], p["a0"][:, 6144:8192])
            hh = half % 2
            for xh in range(2):
                pt = ppool.tile([128, TCOL], F32, tag="ps",
                                padded_shape=[128, TCOL],
                                name=f"pin_{bl}_{xh}")
                for sub in range(2):
                    rhs = a0v[g * 32 : (g + 1) * 32, hh,
                              xh * 32 + sub * 16 : xh * 32 + (sub + 1) * 16, :]
                    nc.tensor.matmul(
                        pt[:, sub * 512 : (sub + 1) * 512],
                        fin_t[g * 32 : (g + 1) * 32, :],
                        rhs,
                        start=True, stop=True,
                        tile_position=(g * 32, 0),
                    )
                evict(
                    X2d[:, (bl * 64 + xh * 32) * 32 : (bl * 64 + (xh + 1) * 32) * 32],
                    pt[:], bin_t[:, 0:1],
                )

        # ---------------- levels 1..5 (q-scheme) ----------------
        cur, cur_nodes = X, 1
        tags = ["s1", "s0", "s1", "s0", "s1"]
        for lvl in range(1, 6):
            n2 = LVL_NODES[lvl - 1]
            grid = int(np.sqrt(n2))
            Hin = LVL_HIN[lvl - 1]
            Ho, W2o = Hin // 2, Hin // 4
            cpn = BG * Ho * W2o          # psum cols per node
            pgrid = int(np.sqrt(cur_nodes))
            nxt = apool.tile([128, n2 * cpn], BF16, tag=tags[lvl - 1],
                             name=f"a{lvl}")
            nxt2d = nxt[:]
            curv = cur[:].rearrange("p (n b h w) -> p n b h w",
                                    n=cur_nodes, b=BG, h=Hin)

            def parent(node):
                if lvl > KLVL:
                    return node
                u, v = node // grid, node % grid
                return (u // 2) * pgrid + (v // 2)

            if cpn >= TCOL:
                # large nodes: tiles split along b (L1: 8/node, L2: 2/node)
                tpn = cpn // TCOL
                bper = TCOL // (Ho * W2o)
                for node in range(n2):
                    pn = parent(node)
                    wsl = w_t[lvl][:]
                    bh = bper // 2       # b per 512-col half
                    for t in range(tpn):
                        bs = t * bper
                        pt = ppool.tile([128, TCOL], F32, tag="ps",
                                        padded_shape=[128, TCOL],
                                        name=f"p{lvl}_{node}_{t}")
                        for x in (0, 1):
                            for q in (0, 1):
                                for sub in (0, 1):
                                    b0 = bs + sub * bh
                                    rhs = curv[:, pn, b0 : b0 + bh, x::2, q::2]
                                    nc.tensor.matmul(
                                        pt[q * 64 : (q + 1) * 64,
                                           sub * 512 : (sub + 1) * 512],
                                        wsl[:, node * 128 + x * 64 :
                                            node * 128 + (x + 1) * 64],
                                        rhs,
                                        start=(x == 0), stop=(x == 1),
                                        skip_group_check=True,
                                        tile_position=(0, q * 64),
                                    )
                        evict(
                            nxt2d[:, (node * BG + bs) * Ho * W2o :
                                  (node * BG + bs + bper) * Ho * W2o],
                            pt[:],
                            bias_t[lvl][:, node : node + 1],
                        )
            else:
                # small nodes: multiple nodes per psum tile
                npt = TCOL // cpn        # L3: 2, L4: 8, L5: 32
                w3ch = None
                for n0 in range(0, n2, npt):
                    pt = ppool.tile([128, npt * cpn], F32, tag="ps",
                                    padded_shape=[128, TCOL],
                                    name=f"p{lvl}_{n0}")
                    for ln in range(npt):
                        node = n0 + ln
                        pn = parent(node)
                        if lvl == 3:
                            if node % 8 == 0:
                                w3ch = wpool.tile([128, 1024], BF16, tag="wch",
                                                  name=f"w3c_{node}")
                                nc.sync.dma_start(
                                    w3ch[:],
                                    p["w3"][:, node * 128 : (node + 8) * 128],
                                )
                            wof = (node % 8) * 128
                            wsl = w3ch
                        else:
                            wof = node * 128
                            wsl = w_t[lvl]
                        for x in (0, 1):
                            for q in (0, 1):
                                rhs = curv[:, pn, :, x::2, q::2]
                                nc.tensor.matmul(
                                    pt[q * 64 : (q + 1) * 64,
                                       ln * cpn : (ln + 1) * cpn],
                                    wsl[:, wof + x * 64 : wof + (x + 1) * 64],
                                    rhs,
                                    start=(x == 0), stop=(x == 1),
                                    skip_group_check=True,
                                    tile_position=(0, q * 64),
                                )
                    for ln in range(npt):
                        node = n0 + ln
                        evict(
                            nxt2d[:, node * cpn : (node + 1) * cpn],
                            pt[:, ln * cpn : (ln + 1) * cpn],
                            bias_t[lvl][:, node : node + 1],
                        )
            cur, cur_nodes = nxt, n2

        # ---------------- level 6 (node pairs, 1x1 out) ----------------
        # feats F2 [128=(sib,c), (pair, b)]
        cur5v = cur[:].rearrange("p (n b h w) -> p n b h w", n=64, b=BG, h=2)
        F2 = fpool.tile([128, 32 * BG], BF16, tag="feats", name="feats")
        F2v = F2[:].rearrange("p (r b) -> p r b", r=32)
        for p0 in range(0, 32, 16):
            pt6 = ppool.tile([128, 16 * BG], F32, tag="ps",
                             padded_shape=[128, TCOL], name=f"p6_{p0}")
            for pr in range(p0, p0 + 16):
                lp = pr - p0
                for x in (0, 1):
                    for half in (0, 1):
                        node = 2 * pr + half
                        rhs = cur5v[:, node, :, x, 0]
                        nc.tensor.matmul(
                            pt6[half * 64 : (half + 1) * 64,
                                lp * BG : (lp + 1) * BG],
                            w_t[6][:, node * 128 + x * 64 :
                                   node * 128 + (x + 1) * 64],
                            rhs,
                            start=(x == 0), stop=(x == 1),
                            skip_group_check=True,
                            tile_position=(0, half * 64),
                        )
            for pr in range(p0, p0 + 16):
                lp = pr - p0
                evict(F2v[:, pr, :], pt6[:, lp * BG : (lp + 1) * BG],
                      bias_t[6][:, pr : pr + 1])

        # ---------------- dense (row-tiled 2x) ----------------
        # separate psum tiles per row-tile parity: two row tiles must not
        # write the same psum bank concurrently
        t2s = fpool.tile([128, NK * NK * BG], F32, tag="t2s", name="t2s")
        t2sv = t2s[:].rearrange("p (n b) -> p n b", n=NK * NK)
        ptd = [
            ppool.tile([128, 512], F32, tag="ps",
                       padded_shape=[128, TCOL], name=f"pd_{s}")
            for s in range(2)
        ]
        for node in range(64):
            k, s = node // 2, node % 2
            rhs = F2v[s * 64 : (s + 1) * 64, k, :]
            nc.tensor.matmul(
                ptd[s][:, k * BG : (k + 1) * BG],
                wd_t[s * 64 : (s + 1) * 64, k * 128 : (k + 1) * 128],
                rhs,
                start=True, stop=True,
                tile_position=(s * 64, 0),
            )
        for s in range(2):
            pv = ptd[s][:].rearrange("p (n b) -> p n b", n=32)
            if s == 0:
                nc.scalar.copy(t2sv[:, 0::2, :], pv[:, :, :])
            else:
                nc.vector.tensor_copy(t2sv[:, 1::2, :], pv[:, :, :])
        nc.sync.dma_start(t2[:], t2s[:])
    nc.compile()
    return nc


# ----------------------------------------------------------------------------
# entry point
# ----------------------------------------------------------------------------

def kernel(**inputs):
    inputs = {k: np.asarray(v) for k, v in inputs.items()}
    wblobs = _prep_weights(inputs)
    nc = _build_kernel()
    in_maps = []
    for c in range(NCORES):
        m = dict(wblobs)
        m["a0"] = _prep_input(inputs["in_data"][c * BC : (c + 1) * BC])
        in_maps.append(m)
    res = run_bass_kernel_spmd(nc, in_maps, list(range(NCORES)))
    outs = [_decode_output(res.results[c]["t2"]) for c in range(NCORES)]
    return np.concatenate(outs, axis=0).astype(np.float32)


if __name__ == "__main__":
    import reference as ref

    inputs = {k: np.asarray(v) for k, v in ref.setup_inputs().items()}
    expected = np.asarray(ref.reference(**inputs))
    actual = kernel(**inputs)
    err = np.abs(actual - expected).max()
    rel = err / np.abs(expected).max()
    print("absmax:", err, "rel:", rel)


# revision 14
# speedup vs baseline: 1.4539x; 1.0562x over previous
"""Trainium2 Bass kernel for nn_ButterflyLayer2D (butterfly 2D CNN).

Strategy: pure data parallel over 8 NeuronCores (16 batch each).

Layouts (per core):
  - activations in SBUF as [128 = (w%2)*64 + c, (node, b, h, w//2)]; every
    2x2-stride-2 per-node conv is 4 bf16 matmuls with K=128=(y,ci):
    q = output w-parity goes to PE column-tile (0, q*64) so the two q
    streams run CONCURRENTLY on the two column halves of the PE array,
    x = input h-parity accumulates in PSUM.  PSUM rows are (q, c_out) ==
    exactly the next level's partition layout, so each eviction is ONE
    full-width relu(+bias) op with contiguous psum read and contiguous
    sbuf write (alternating ScalarE/VectorE).
  - the input 4x4-patch conv packs TWO horizontally adjacent patches per
    rhs column (K=32, block-diagonal lhsT, M=128=(w%2, c)) and row-tiles
    4 ways over (b%4); it is interleaved with level 1 per batch-pair so
    the eviction engines never idle.
  - level 6 (spatial 1x1) computes node pairs with M=(coA,coB);
    features land as [128=(sib,c), (pair, b)].
  - the final dense is row-tiled 2x (even nodes PE rows 0-63, odd 64-127)
    into parity-major psum tiles; output written parity-major and
    decoded on the host.
When all biases are zero (checked on host) psum tiles covering several
nodes are evicted in one op; otherwise per-node evicts apply the bias.
Weights are prefetched whole-level across both HWDGE rings; w3 streams
just-in-time in 8-node chunks.
"""

import numpy as np
from contextlib import ExitStack

import concourse.bass as bass
import concourse.tile as tile
from concourse import bacc, mybir
from concourse.bass_utils import run_bass_kernel_spmd

F32 = mybir.dt.float32
BF16 = mybir.dt.bfloat16
AF = mybir.ActivationFunctionType
ALU = mybir.AluOpType

B, IN, NLVL, KLVL, C = 128, 256, 6, 3, 64
NK, OU, OV = 8, 8, 8
NCORES = 8
BC = B // NCORES          # 16 per-core batch
PH = 1
BG = BC
TCOL = 1024               # psum tile columns (2 banks)
PBUFS = 4
LVL_NODES = [4, 16, 64, 64, 64, 64]
LVL_HIN = [64, 32, 16, 8, 4, 2]


# ----------------------------------------------------------------------------
# host-side pre-arrangement
# ----------------------------------------------------------------------------

def _prep_weights(inputs):
    """Weights/biases blobs shared by all cores."""
    import ml_dtypes
    out = {}
    # input filter: block-diag lhsT [32=(yp,pix), 128=(yp_out, c)], repl x4
    fin = inputs["in_filter"][:, :, 0, :].reshape(16, C).astype(np.float32)
    blk = np.zeros((32, 128), np.float32)
    blk[0:16, 0:64] = fin
    blk[16:32, 64:128] = fin
    finr = np.zeros((128, 128), np.float32)
    for g in range(4):
        finr[g * 32 : (g + 1) * 32] = blk
    out["fin"] = finr.astype(ml_dtypes.bfloat16)
    out["bin"] = np.concatenate([inputs["in_bias"], inputs["in_bias"]]).reshape(
        128, 1
    ).astype(np.float32)

    for lvl in range(1, NLVL + 1):
        f = inputs[f"f{lvl}"].astype(np.float32)  # [n,n,2,2,C,C] (x,y,ci,co)
        n = f.shape[0]
        # per node lhsT [(y,ci)=128, (x,co)=128]
        w = f.transpose(0, 1, 3, 4, 2, 5).reshape(n * n, 2 * C, 2 * C)
        out[f"w{lvl}"] = np.ascontiguousarray(w.transpose(1, 0, 2)).reshape(
            128, n * n * 128
        ).astype(ml_dtypes.bfloat16)
        b = inputs[f"b{lvl}"].astype(np.float32).reshape(n * n, C)
        if lvl < NLVL:
            bb = np.concatenate([b, b], axis=1)  # [nodes, (q,c)=128]
            out[f"b{lvl}"] = np.ascontiguousarray(bb.T)
        else:
            bb = b.reshape(n * n // 2, 2 * C)    # [pairs, (cA,cB)]
            out[f"b{lvl}"] = np.ascontiguousarray(bb.T)  # [128, 32]
    # dense: [128, pairs*128]: rows 0-63 even-node [c,(r,ouov)], 64-127 odd
    wd = inputs["Wd"].astype(np.float32).reshape(NK * NK, 2, C, OU * OV)
    wd = wd.transpose(2, 0, 1, 3).reshape(C, NK * NK, 2 * OU * OV)
    wd2 = np.zeros((128, NK * NK // 2 * 128), np.float32)
    for k in range(NK * NK // 2):
        wd2[0:64, k * 128 : (k + 1) * 128] = wd[:, 2 * k]
        wd2[64:128, k * 128 : (k + 1) * 128] = wd[:, 2 * k + 1]
    out["wd"] = wd2.astype(ml_dtypes.bfloat16)
    return out


def _prep_input(in_data_core):
    """Per-core input blob [128 = (b%4)*32 + (j%8//4)*16 + (i%4)*4 + (j%4),
    (b//4, x=i//4, y2=j//8)]."""
    import ml_dtypes
    ind = in_data_core[:, :, :, 0]  # [16, 256, 256]
    a = ind.reshape(4, 4, 64, 4, 32, 2, 4)  # [half, g, x, p, y2, yp, q]
    a = a.transpose(1, 5, 3, 6, 0, 2, 4)    # [g, yp, p, q, half, x, y2]
    return np.ascontiguousarray(a).reshape(128, 4 * 64 * 32).astype(
        ml_dtypes.bfloat16
    )


def _decode_output(t2_core):
    """t2 [128=(r,ou,ov), (s, k, b)] with node = 2k+s -> [16, 64, 64, 2]."""
    t = t2_core.reshape(2, OU, OV, 2, 8, 4, BG)  # r,ou,ov,s,u,v2,b
    t = t.transpose(6, 4, 1, 5, 3, 2, 0)         # b,u,ou,v2,s,ov,r
    return np.ascontiguousarray(t).reshape(BC, NK * OU, NK * OV, 2)


# ----------------------------------------------------------------------------
# device kernel
# ----------------------------------------------------------------------------

def _build_kernel(zero_bias=True):
    nc = bacc.Bacc(None, target_bir_lowering=False)
    p = {}
    p["a0"] = nc.declare_dram_parameter("a0", [128, 4 * 64 * 32], BF16, isOutput=False)
    p["fin"] = nc.declare_dram_parameter("fin", [128, 128], BF16, isOutput=False)
    p["bin"] = nc.declare_dram_parameter("bin", [128, 1], F32, isOutput=False)
    for lvl in range(1, NLVL + 1):
        n2 = LVL_NODES[lvl - 1]
        p[f"w{lvl}"] = nc.declare_dram_parameter(f"w{lvl}", [128, n2 * 128], BF16, isOutput=False)
        bcols = n2 if lvl < NLVL else n2 // 2
        p[f"b{lvl}"] = nc.declare_dram_parameter(f"b{lvl}", [128, bcols], F32, isOutput=False)
    p["wd"] = nc.declare_dram_parameter("wd", [128, NK * NK // 2 * 128], BF16, isOutput=False)
    t2 = nc.declare_dram_parameter("t2", [128, NK * NK * BG], F32, isOutput=True)

    evict_ctr = [0]

    def evict(out_ap, psum_ap, bias_ap=None):
        """relu(psum [+ bias]) -> sbuf, alternating engines."""
        evict_ctr[0] += 1
        if evict_ctr[0] % 2 == 0:
            if bias_ap is None:
                nc.scalar.activation(out_ap, psum_ap, AF.Relu)
            else:
                nc.scalar.activation(out_ap, psum_ap, AF.Relu, bias=bias_ap)
        elif bias_ap is None:
            nc.vector.tensor_scalar_max(out_ap, psum_ap, 0.0)
        else:
            nc.vector.tensor_scalar(out_ap, psum_ap, bias_ap, 0.0,
                                    op0=ALU.add, op1=ALU.max)

    with tile.TileContext(nc) as tc, ExitStack() as ctx:
        const = ctx.enter_context(tc.tile_pool(name="const", bufs=1))
        wbig = ctx.enter_context(tc.tile_pool(name="wbig", bufs=1))
        wpool = ctx.enter_context(tc.tile_pool(name="wts", bufs=2))
        apool = ctx.enter_context(tc.tile_pool(name="acts", bufs=1))
        inpool = ctx.enter_context(tc.tile_pool(name="inp", bufs=1))
        fpool = ctx.enter_context(tc.tile_pool(name="feat", bufs=1))
        ppool = ctx.enter_context(tc.tile_pool(name="ps", bufs=PBUFS, space="PSUM"))

        # ---------------- DMA prologue ----------------
        # scalar ring (fast q10): a0 halves 0,1 then big weights w5,w6,wd,w4
        # sync ring: consts, w1, w2; later a0 halves 2,3 + w3 chunks in-loop
        a0s = inpool.tile([128, 2 * 64 * 32], BF16, tag="a0s", name="a0s")
        nc.scalar.dma_start(a0s[:, 0:2048], p["a0"][:, 0:2048])
        nc.scalar.dma_start(a0s[:, 2048:4096], p["a0"][:, 2048:4096])
        fin_t = const.tile([128, 128], BF16)
        nc.sync.dma_start(fin_t[:], p["fin"][:])
        bin_t = const.tile([128, 1], F32)
        nc.sync.dma_start(bin_t[:], p["bin"][:])
        bias_t = {}
        for lvl in range(1, NLVL + 1):
            bcols = LVL_NODES[lvl - 1] if lvl < NLVL else LVL_NODES[lvl - 1] // 2
            bias_t[lvl] = const.tile([128, bcols], F32, tag=f"bias{lvl}", name=f"bias{lvl}")
            nc.sync.dma_start(bias_t[lvl][:], p[f"b{lvl}"][:])
        w_t = {}
        for lvl in (1, 2):
            n2 = LVL_NODES[lvl - 1]
            w_t[lvl] = wbig.tile([128, n2 * 128], BF16, tag=f"w{lvl}", name=f"w{lvl}")
            nc.sync.dma_start(w_t[lvl][:], p[f"w{lvl}"][:])
        for lvl in (5, 6):
            w_t[lvl] = wbig.tile([128, 64 * 128], BF16, tag=f"w{lvl}", name=f"w{lvl}")
            nc.scalar.dma_start(w_t[lvl][:], p[f"w{lvl}"][:])
        wd_t = wbig.tile([128, NK * NK // 2 * 128], BF16, tag="wd", name="wd")
        nc.scalar.dma_start(wd_t[:], p["wd"][:])
        w_t[4] = wbig.tile([128, 64 * 128], BF16, tag="w4", name="w4")
        nc.scalar.dma_start(w_t[4][:], p["w4"][:])

        a0v = a0s[:].rearrange("p (h x y) -> p h x y", h=2, x=64)

        # ---------------- input conv + level 1, interleaved ----------------
        # X: [128=(w%2,c), (b, h=64, w2=32)]; L1 out: [128, (n=4, b, 32, 16)]
        X = apool.tile([128, BG * 64 * 32], BF16, tag="s0", name="x0")
        X2d = X[:]
        Xv = X[:].rearrange("p (b h w) -> p b h w", b=BG, h=64)
        a1 = apool.tile([128, 4 * BG * 32 * 16], BF16, tag="s1", name="a1")
        a12d = a1[:]
        for pr in range(8):
            # stage the next a0 slot as soon as the previous pair freed it
            if pr == 2:
                nc.sync.dma_start(a0s[:, 0:2048], p["a0"][:, 4096:6144])
            elif pr == 4:
                nc.sync.dma_start(a0s[:, 2048:4096], p["a0"][:, 6144:8192])
            for bl in (2 * pr, 2 * pr + 1):
                g, hh = bl % 4, (bl // 4) % 2
                for xh in range(2):
                    pt = ppool.tile([128, TCOL], F32, tag="ps",
                                    padded_shape=[128, TCOL],
                                    name=f"pin_{bl}_{xh}")
                    for sub in range(2):
                        rhs = a0v[g * 32 : (g + 1) * 32, hh,
                                  xh * 32 + sub * 16 : xh * 32 + (sub + 1) * 16, :]
                        nc.tensor.matmul(
                            pt[:, sub * 512 : (sub + 1) * 512],
                            fin_t[g * 32 : (g + 1) * 32, :],
                            rhs,
                            start=True, stop=True,
                            tile_position=(g * 32, 0),
                        )
                    evict(
                        X2d[:, (bl * 64 + xh * 32) * 32 :
                            (bl * 64 + (xh + 1) * 32) * 32],
                        pt[:], bin_t[:, 0:1],
                    )
            # L1 for this batch pair (4 nodes; parent is node 0 == whole X)
            bs = 2 * pr
            for node in range(4):
                pt = ppool.tile([128, TCOL], F32, tag="ps",
                                padded_shape=[128, TCOL],
                                name=f"p1_{node}_{pr}")
                for x in (0, 1):
                    for q in (0, 1):
                        for sub in (0, 1):
                            rhs = Xv[:, bs + sub, x::2, q::2]
                            nc.tensor.matmul(
                                pt[q * 64 : (q + 1) * 64,
                                   sub * 512 : (sub + 1) * 512],
                                w_t[1][:, node * 128 + x * 64 :
                                       node * 128 + (x + 1) * 64],
                                rhs,
                                start=(x == 0), stop=(x == 1),
                                skip_group_check=True,
                                tile_position=(0, q * 64),
                            )
                evict(
                    a12d[:, (node * BG + bs) * 512 : (node * BG + bs + 2) * 512],
                    pt[:],
                    bias_t[1][:, node : node + 1],
                )

        # ---------------- levels 2..5 (q-scheme) ----------------
        cur, cur_nodes = a1, 4
        tags = ["s0", "s1", "s0", "s1"]
        for lvl in range(2, 6):
            n2 = LVL_NODES[lvl - 1]
            grid = int(np.sqrt(n2))
            Hin = LVL_HIN[lvl - 1]
            Ho, W2o = Hin // 2, Hin // 4
            cpn = BG * Ho * W2o          # psum cols per node
            pgrid = int(np.sqrt(cur_nodes))
            nxt = apool.tile([128, n2 * cpn], BF16, tag=tags[lvl - 2],
                             name=f"a{lvl}")
            nxt2d = nxt[:]
            curv = cur[:].rearrange("p (n b h w) -> p n b h w",
                                    n=cur_nodes, b=BG, h=Hin)

            def parent(node):
                if lvl > KLVL:
                    return node
                u, v = node // grid, node % grid
                return (u // 2) * pgrid + (v // 2)

            if cpn >= TCOL:
                # large nodes (L2: 2 tiles/node split along b)
                tpn = cpn // TCOL
                bper = TCOL // (Ho * W2o)
                bh = bper // 2
                for node in range(n2):
                    pn = parent(node)
                    for t in range(tpn):
                        bs = t * bper
                        pt = ppool.tile([128, TCOL], F32, tag="ps",
                                        padded_shape=[128, TCOL],
                                        name=f"p{lvl}_{node}_{t}")
                        for x in (0, 1):
                            for q in (0, 1):
                                for sub in (0, 1):
                                    b0 = bs + sub * bh
                                    rhs = curv[:, pn, b0 : b0 + bh, x::2, q::2]
                                    nc.tensor.matmul(
                                        pt[q * 64 : (q + 1) * 64,
                                           sub * 512 : (sub + 1) * 512],
                                        w_t[lvl][:, node * 128 + x * 64 :
                                                 node * 128 + (x + 1) * 64],
                                        rhs,
                                        start=(x == 0), stop=(x == 1),
                                        skip_group_check=True,
                                        tile_position=(0, q * 64),
                                    )
                        evict(
                            nxt2d[:, (node * BG + bs) * Ho * W2o :
                                  (node * BG + bs + bper) * Ho * W2o],
                            pt[:],
                            bias_t[lvl][:, node : node + 1],
                        )
            else:
                # small nodes: multiple nodes per psum tile
                npt = TCOL // cpn        # L3: 2, L4: 8, L5: 32
                w3ch = None
                for n0 in range(0, n2, npt):
                    pt = ppool.tile([128, npt * cpn], F32, tag="ps",
                                    padded_shape=[128, TCOL],
                                    name=f"p{lvl}_{n0}")
                    for ln in range(npt):
                        node = n0 + ln
                        pn = parent(node)
                        if lvl == 3:
                            if node % 8 == 0:
                                w3ch = wpool.tile([128, 1024], BF16, tag="wch",
                                                  name=f"w3c_{node}")
                                nc.sync.dma_start(
                                    w3ch[:],
                                    p["w3"][:, node * 128 : (node + 8) * 128],
                                )
                            wof = (node % 8) * 128
                            wsl = w3ch
                        else:
                            wof = node * 128
                            wsl = w_t[lvl]
                        for x in (0, 1):
                            for q in (0, 1):
                                rhs = curv[:, pn, :, x::2, q::2]
                                nc.tensor.matmul(
                                    pt[q * 64 : (q + 1) * 64,
                                       ln * cpn : (ln + 1) * cpn],
                                    wsl[:, wof + x * 64 : wof + (x + 1) * 64],
                                    rhs,
                                    start=(x == 0), stop=(x == 1),
                                    skip_group_check=True,
                                    tile_position=(0, q * 64),
                                )
                    if zero_bias:
                        evict(nxt2d[:, n0 * cpn : (n0 + npt) * cpn],
                              pt[:, : npt * cpn])
                    else:
                        for ln in range(npt):
                            node = n0 + ln
                            evict(
                                nxt2d[:, node * cpn : (node + 1) * cpn],
                                pt[:, ln * cpn : (ln + 1) * cpn],
                                bias_t[lvl][:, node : node + 1],
                            )
            cur, cur_nodes = nxt, n2

        # ---------------- level 6 (node pairs, 1x1 out) ----------------
        # feats F2 [128=(sib,c), (pair, b)]
        cur5v = cur[:].rearrange("p (n b h w) -> p n b h w", n=64, b=BG, h=2)
        F2 = fpool.tile([128, 32 * BG], BF16, tag="feats", name="feats")
        F2v = F2[:].rearrange("p (r b) -> p r b", r=32)
        for p0 in range(0, 32, 16):
            pt6 = ppool.tile([128, 16 * BG], F32, tag="ps",
                             padded_shape=[128, TCOL], name=f"p6_{p0}")
            for pr in range(p0, p0 + 16):
                lp = pr - p0
                for x in (0, 1):
                    for half in (0, 1):
                        node = 2 * pr + half
                        rhs = cur5v[:, node, :, x, 0]
                        nc.tensor.matmul(
                            pt6[half * 64 : (half + 1) * 64,
                                lp * BG : (lp + 1) * BG],
                            w_t[6][:, node * 128 + x * 64 :
                                   node * 128 + (x + 1) * 64],
                            rhs,
                            start=(x == 0), stop=(x == 1),
                            skip_group_check=True,
                            tile_position=(0, half * 64),
                        )
            if zero_bias:
                evict(F2[:][:, p0 * BG : (p0 + 16) * BG], pt6[:])
            else:
                for pr in range(p0, p0 + 16):
                    lp = pr - p0
                    evict(F2v[:, pr, :], pt6[:, lp * BG : (lp + 1) * BG],
                          bias_t[6][:, pr : pr + 1])

        # ---------------- dense (row-tiled 2x, parity-major out) ----------
        t2s = fpool.tile([128, NK * NK * BG], F32, tag="t2s", name="t2s")
        ptd = [
            ppool.tile([128, 512], F32, tag="ps",
                       padded_shape=[128, TCOL], name=f"pd_{s}")
            for s in range(2)
        ]
        for node in range(64):
            k, s = node // 2, node % 2
            rhs = F2v[s * 64 : (s + 1) * 64, k, :]
            nc.tensor.matmul(
                ptd[s][:, k * BG : (k + 1) * BG],
                wd_t[s * 64 : (s + 1) * 64, k * 128 : (k + 1) * 128],
                rhs,
                start=True, stop=True,
                tile_position=(s * 64, 0),
            )
        nc.scalar.copy(t2s[:, 0:512], ptd[0][:])
        nc.scalar.dma_start(t2[:, 0:512], t2s[:, 0:512])
        nc.vector.tensor_copy(t2s[:, 512:1024], ptd[1][:])
        nc.scalar.dma_start(t2[:, 512:1024], t2s[:, 512:1024])
    nc.compile()
    return nc


# ----------------------------------------------------------------------------
# entry point
# ----------------------------------------------------------------------------

def kernel(**inputs):
    inputs = {k: np.asarray(v) for k, v in inputs.items()}
    zb = not np.any(inputs["in_bias"]) and all(
        not np.any(inputs[f"b{l}"]) for l in range(1, NLVL + 1)
    )
    wblobs = _prep_weights(inputs)
    nc = _build_kernel(zero_bias=zb)
    in_maps = []
    for c in range(NCORES):
        m = dict(wblobs)
        m["a0"] = _prep_input(inputs["in_data"][c * BC : (c + 1) * BC])
        in_maps.append(m)
    res = run_bass_kernel_spmd(nc, in_maps, list(range(NCORES)))
    outs = [_decode_output(res.results[c]["t2"]) for c in range(NCORES)]
    return np.concatenate(outs, axis=0).astype(np.float32)


if __name__ == "__main__":
    import reference as ref

    inputs = {k: np.asarray(v) for k, v in ref.setup_inputs().items()}
    expected = np.asarray(ref.reference(**inputs))
    actual = kernel(**inputs)
    err = np.abs(actual - expected).max()
    rel = err / np.abs(expected).max()
    print("absmax:", err, "rel:", rel)
